# revision 28
# baseline (speedup 1.0000x reference)
"""Block self-attention (chunked, q=k=v, no projections) on 8 Trainium2 cores.

Math (per reference): x:[B,S,D] -> [B,H,S,dh] -> chunks of 256 along S ->
per (b,chunk,head): A = x_chunk [256,64]; S = A@A.T/8; P = softmax(S);
O = P@A -> reassembled to [B,S,D].

Device-side structure (ACT-bound design, fp16 datapath):
  * The host pre-transposes x into the [d, seq] layout the PE needs, so the
    kernel does NO on-chip input transposes of x and no PSUM->SBUF copies
    of them.
  * S is symmetric per head, so only the upper blocks [S00|S01|S11] are
    computed and exp'd on the ACT engine (3/4 of the elements; ACT is the
    bottleneck engine).  The missing lower block E10 = E01^T is recovered
    with one fp16 PE transpose per head + a DVE copy back to SBUF.
  * The PV matmul's moving operand is a host-prebuilt fp16 tile with a ones
    column appended per head; the ones column makes the PV matmul also emit
    the softmax denominator (row sum of E).
  * exp() runs as exp(score/8 + bias) with a per-(chunk,head-pair) bias
    computed on the host from row norms (Cauchy-Schwarz bound); the shift
    cancels exactly in the normalization.
  * Output is written as fp16 (halves output DMA); the host casts to fp32.
  * Engine-stream emission order per pair-iteration P:
    PE [mm1(P), mm2(P-2), E10transpose(P-1)] so the only ACT-dependent PE
    instruction sits at the END of the iteration's PE work, keeping the
    stream dense while ACT (the critical path) stays saturated.

Sharding: data-parallel over the fused (batch*chunk) dim: 64 chunks total,
8 consecutive chunks per core == one contiguous [2048, 1024] row-slice of
the flattened [16384, 1024] input per core.
"""

import numpy as np

B, S, D = 4, 4096, 1024
H = 16
DH = D // H              # 64
CHUNK = 256
NCORES = 8
NPAIR = H // 2           # 8 head pairs
ROWS_PER_CORE = (B * S) // NCORES         # 2048
CHUNKS_PER_CORE = ROWS_PER_CORE // CHUNK  # 8
SCALE = 1.0 / 8.0        # 1/sqrt(dh)
GW = DH + 1              # per-head group width in the ones-augmented operand
# exp output stays well inside fp16 range and above its subnormals:
# ln(30000) ~ 10.3 of headroom below fp16 max 65504.
EXP_MARGIN = float(np.log(30000.0))

USE_SYM = True      # block-symmetric exp (3 of 4 blocks) + E10 transpose

_PROGRAM = None


def _build_program():
    import concourse.bass as bass
    import concourse.tile as tile
    from concourse import bacc, mybir
    from concourse.masks import make_identity

    f32 = mybir.dt.float32
    f16 = mybir.dt.float16
    Exp = mybir.ActivationFunctionType.Exp

    nc = bacc.Bacc("TRN2", target_bir_lowering=False, debug=False,
                   num_devices=NCORES)
    # xt: host-transposed input. Row (c*128+p), col (hp*256+s) holds
    # x[c*256+s, (2hp + p//64)*64 + p%64] for chunk c of this core.
    xt = nc.dram_tensor("xt", [CHUNKS_PER_CORE * 128, NPAIR * CHUNK], f16,
                        kind="ExternalInput")
    # xdr: interleaved+ones PV moving operand. Row (c*128+p), col
    # (i*H*GW + h*GW + dd) holds x[c*256 + i*128 + p, h*64+dd] (dd<64) or 1.
    xdr = nc.dram_tensor("xdr", [CHUNKS_PER_CORE * 128, 2 * H * GW], f16,
                         kind="ExternalInput")
    # eb: per-(chunk, pair) exp bias, replicated across partitions.
    eb = nc.dram_tensor("eb", [128, CHUNKS_PER_CORE * NPAIR], f32,
                        kind="ExternalInput")
    y = nc.dram_tensor("y", [ROWS_PER_CORE, D], f16, kind="ExternalOutput")
    xtap = xt.ap()
    xdap = xdr.ap()
    yap = y.ap()

    # e-tile layout per head (stride 512): [B00 | B01 | B11 | B10t] (sym)
    # or [B00 | B01 | B10 | B11] (full), each block [128,128].
    with tile.TileContext(nc) as tc:
        with (
            tc.tile_pool(name="const", bufs=1) as const_pool,
            tc.tile_pool(name="xt", bufs=12) as xt_pool,
            tc.tile_pool(name="xd", bufs=3) as xd_pool,
            tc.tile_pool(name="sc", bufs=2, space="PSUM") as sc_pool,
            tc.tile_pool(name="et", bufs=2, space="PSUM") as et_pool,
            tc.tile_pool(name="o", bufs=1, space="PSUM") as o_pool,
            tc.tile_pool(name="e8", bufs=6) as e_pool,
            tc.tile_pool(name="rc", bufs=8) as r_pool,
            tc.tile_pool(name="y", bufs=3) as y_pool,
        ):
            ebias = const_pool.tile([128, CHUNKS_PER_CORE * NPAIR], f32)
            nc.sync.dma_start(out=ebias[:], in_=eb.ap())
            ident = const_pool.tile([128, 128], f16)

            def emit_front(c, hp, xt_t):
                # mm1 (upper blocks) + exp for pair (c, hp); xt_t is the
                # 2-pair [128, 512] slice tile holding pairs (hp&~1, hp|1).
                s_ps = sc_pool.tile([128, 1024], f32, tag="sc",
                                    name=f"sc{c}_{hp}")
                x0 = (hp % 2) * CHUNK
                for hi in range(2):
                    lt = xt_t[64 * hi:64 * hi + 64, :]
                    col = hi * 512
                    if USE_SYM:
                        # [S00|S01] (q0 x all k), then S11.
                        nc.tensor.matmul(
                            out=s_ps[:, col:col + 256],
                            lhsT=lt[:, x0:x0 + 128], rhs=lt[:, x0:x0 + 256],
                            start=True, stop=True)
                        nc.tensor.matmul(
                            out=s_ps[:, col + 256:col + 384],
                            lhsT=lt[:, x0 + 128:x0 + 256],
                            rhs=lt[:, x0 + 128:x0 + 256],
                            start=True, stop=True)
                    else:
                        for qm in range(2):
                            nc.tensor.matmul(
                                out=s_ps[:, col + qm * 256:col + qm * 256 + 256],
                                lhsT=lt[:, x0 + qm * 128:x0 + qm * 128 + 128],
                                rhs=lt[:, x0:x0 + 256],
                                start=True, stop=True)

                e8 = e_pool.tile([128, 1024], f16, tag="e", name=f"e{c}_{hp}")
                ncols = 384 if USE_SYM else 512
                e_out = bass.AP(tensor=e8.tensor, offset=e8.offset,
                                ap=[e8.ap[0], [512, 2], [1, ncols]])
                s_in = bass.AP(tensor=s_ps.tensor, offset=s_ps.offset,
                               ap=[s_ps.ap[0], [512, 2], [1, ncols]])
                nc.scalar.activation(out=e_out, in_=s_in, func=Exp,
                                     scale=SCALE,
                                     bias=ebias[:, c * NPAIR + hp:
                                                c * NPAIR + hp + 1])
                return e8

            def emit_mid(c, hp, e8):
                # E10 = E01^T via PE transpose, copied back into the e-tile.
                if not USE_SYM:
                    return
                et_ps = et_pool.tile([128, 256], f16, tag="et",
                                     name=f"et{c}_{hp}")
                for hi in range(2):
                    nc.tensor.matmul(
                        out=et_ps[:, hi * 128:hi * 128 + 128],
                        lhsT=e8[:, hi * 512 + 128:hi * 512 + 256],
                        rhs=ident[:], is_transpose=True,
                        start=True, stop=True)
                dst = bass.AP(tensor=e8.tensor, offset=e8.offset + 384,
                              ap=[e8.ap[0], [512, 2], [1, 128]])
                src = bass.AP(tensor=et_ps.tensor, offset=et_ps.offset,
                              ap=[et_ps.ap[0], [128, 2], [1, 128]])
                nc.vector.tensor_copy(out=dst, in_=src)

            def emit_back2(st0, st1):
                # PV matmuls (+ ones-column denominators) for TWO pairs into
                # one 2-bank PSUM tile, then ONE reciprocal and ONE
                # normalization multiply covering both pairs (the DVE's
                # ~170ns fixed cost per op is what makes it the
                # co-bottleneck otherwise), then the group's output DMA.
                c, hp0 = st0[0], st0[1]
                o2 = o_pool.tile([128, 1024], f32, tag="o",
                                 name=f"o{c}_{hp0}")
                for ps, (c_, hp, e8, xd_t, yt) in enumerate((st0, st1)):
                    for hi in range(2):
                        h = 2 * hp + hi
                        for b_ in range(2):
                            # o2 col layout: b*512 + ps*130 + hi*65 (+dd);
                            # every matmul window stays inside a 2KB bank.
                            col = b_ * 512 + ps * 130 + hi * 65
                            for i in range(2):
                                if USE_SYM:
                                    w_off = (hi * 512 + b_ * 128 if i == 0
                                             else hi * 512 +
                                             (384 if b_ == 0 else 256))
                                else:
                                    w_off = hi * 512 + i * 256 + b_ * 128
                                nc.tensor.matmul(
                                    out=o2[:, col:col + GW],
                                    lhsT=e8[:, w_off:w_off + 128],
                                    rhs=bass.AP(
                                        tensor=xd_t.tensor,
                                        offset=(xd_t.offset + i * H * GW
                                                + h * GW),
                                        ap=[xd_t.ap[0], [1, GW]]),
                                    start=(i == 0), stop=(i == 1))

                c, hp1, _, _, yt = st1
                rc = r_pool.tile([128, 8], f32, tag="rc", name=f"rc{c}_{hp1}")
                nc.vector.reciprocal(
                    out=rc[:].rearrange("p (b g c) -> p b g c", g=4, c=1),
                    in_=bass.AP(tensor=o2.tensor, offset=o2.offset + DH,
                                ap=[o2.ap[0], [512, 2], [65, 4], [1, 1]]))
                # yt[:, b*1024 + (hp0+ps)*128 + hi*64 + dd] =
                #     o2[:, b*512 + (2ps+hi)*65 + dd] * rc[:, b*4 + 2ps+hi]
                out_v = bass.AP(tensor=yt.tensor,
                                offset=yt.offset + hp0 * 128,
                                ap=[yt.ap[0], [1024, 2], [64, 4], [1, DH]])
                in0 = bass.AP(tensor=o2.tensor, offset=o2.offset,
                              ap=[o2.ap[0], [512, 2], [65, 4], [1, DH]])
                in1 = bass.AP(tensor=rc.tensor, offset=rc.offset,
                              ap=[rc.ap[0], [4, 2], [1, 4], [0, DH]])
                nc.vector.tensor_mul(out_v, in0, in1)
                # stream out this 2-pair column group (256 cols); the very
                # tail group splits across queues/dispatchers so the final
                # transfer drains fast.
                g = hp0 // 2
                splits = 4 if (g == 3 and c == CHUNKS_PER_CORE - 1) \
                    else (2 if g == 3 else 1)
                w = 256 // splits
                for s_ in range(splits):
                    dst = bass.AP(tensor=yap.tensor,
                                  offset=c * CHUNK * D + g * 256 + s_ * w,
                                  ap=[[D, 128], [128 * D, 2], [1, w]])
                    src = bass.AP(tensor=yt.tensor,
                                  offset=yt.offset + g * 256 + s_ * w,
                                  ap=[yt.ap[0], [1024, 2], [1, w]])
                    eng = nc.gpsimd if s_ % 2 else nc.sync
                    eng.dma_start(out=dst, in_=src)

            # Emission order per iteration P: front(P) [PE mm1 + ACT exp],
            # back(P-2) [PE mm2 + DVE], mid(P-1) [PE transpose + DVE copy].
            # mid's transpose is the only PE instruction that waits on ACT,
            # and it sits after the iteration's other PE work.
            stages = []   # (c, hp, e8, xd_t, yt)
            lag = 2 if USE_SYM else 1
            for c in range(CHUNKS_PER_CORE):
                # Input loads ride the (otherwise idle) GpSimd sequencer so
                # Sync's ~650ns-per-dispatch budget stays with the output
                # stream.  xt is loaded as four separate 2-pair tiles: a
                # pair's mm1 then depends only on its own 128KB slice, so
                # the first exp fires ~10us earlier than with one 530KB
                # load-tile dependency.
                xtg = []
                for sl in range(4):
                    w = 2 * CHUNK
                    t = xt_pool.tile([128, w], f16, tag="xt",
                                     name=f"xt{c}_{sl}")
                    nc.gpsimd.dma_start(
                        out=t[:],
                        in_=xtap[c * 128:(c + 1) * 128, sl * w:(sl + 1) * w])
                    xtg.append(t)
                xd_t = xd_pool.tile([128, 2 * H * GW], f16, tag="xd",
                                    name=f"xd{c}")
                for sl in range(2):
                    w = H * GW
                    nc.gpsimd.dma_start(
                        out=xd_t[:, sl * w:(sl + 1) * w],
                        in_=xdap[c * 128:(c + 1) * 128, sl * w:(sl + 1) * w])
                if c == 0:
                    # identity (for the E10 transposes) is first needed a
                    # few pairs in; build it after the chunk-0 dispatches.
                    make_identity(nc, ident[:])
                yt = y_pool.tile([128, 2 * 1024], f16, tag="y", name=f"y{c}")

                for hp in range(NPAIR):
                    P = c * NPAIR + hp
                    e8 = emit_front(c, hp, xtg[hp // 2])
                    stages.append((c, hp, e8, xd_t, yt))
                    # back2 for pair batch (P-3, P-2) at every odd P; the
                    # E10 transpose for pair P-1 goes last in the PE stream.
                    if P % 2 == 1 and P >= 3:
                        emit_back2(stages[-4], stages[-3])
                    if USE_SYM and len(stages) >= 2:
                        emit_mid(*stages[-2][:3])
                    stages = stages[-5:]
            # drain
            if USE_SYM:
                emit_mid(*stages[-1][:3])
            emit_back2(stages[-2], stages[-1])

    nc.compile()
    return nc


def _get_program():
    global _PROGRAM
    if _PROGRAM is None:
        _PROGRAM = _build_program()
    return _PROGRAM


def _reference_numpy(hs, mask):
    # Exact reference math in numpy; only used if a nonzero mask ever shows
    # up (the input spec pins the mask to zeros).
    NC_ = S // CHUNK
    xx = hs.reshape(B, S, H, DH).transpose(0, 2, 1, 3)
    q = xx.reshape(B * NC_, H, CHUNK, DH)
    m = mask.reshape(B * NC_, 1, 1, CHUNK)
    scores = np.einsum('bhqd,bhkd->bhqk', q, q) / np.sqrt(DH) + m
    scores -= scores.max(axis=-1, keepdims=True)
    probs = np.exp(scores)
    probs /= probs.sum(axis=-1, keepdims=True)
    ctx = np.einsum('bhqk,bhkd->bhqd', probs, q)
    return (ctx.reshape(B, H, S, DH).transpose(0, 2, 1, 3)
            .reshape(B, S, D).astype(np.float32))


def _prep_inputs(hs):
    """Host-side layout prep: transposed fp16 operand, interleaved+ones PV
    operand, per-(chunk,pair) exp biases."""
    x16 = hs.astype(np.float16)                       # [B,S,D]
    v = x16.reshape(NCORES, CHUNKS_PER_CORE, CHUNK, H, DH)  # n,c,s,h,d
    # xt[n, c, hi*64+d, hp, s]
    xt = (v.reshape(NCORES, CHUNKS_PER_CORE, CHUNK, NPAIR, 2, DH)
          .transpose(0, 1, 4, 5, 3, 2)               # n,c,hi,d,hp,s
          .reshape(NCORES, CHUNKS_PER_CORE * 128, NPAIR * CHUNK))
    xt = np.ascontiguousarray(xt)
    # xdr[n, c, p, i, h, dd]
    w = v.reshape(NCORES, CHUNKS_PER_CORE, 2, 128, H, DH)
    aug = np.empty((NCORES, CHUNKS_PER_CORE, 2, 128, H, GW), dtype=np.float16)
    aug[..., :DH] = w
    aug[..., DH] = np.float16(1.0)
    xdr = np.ascontiguousarray(
        aug.transpose(0, 1, 3, 2, 4, 5)
        .reshape(NCORES, CHUNKS_PER_CORE * 128, 2 * H * GW))
    # per-(core, chunk, pair) bias from the Cauchy-Schwarz score bound
    n2 = (x16.astype(np.float32) ** 2).reshape(
        NCORES, CHUNKS_PER_CORE, CHUNK, H, DH).sum(-1) * SCALE  # n,c,s,h
    pmax = n2.reshape(NCORES, CHUNKS_PER_CORE, CHUNK, NPAIR, 2).max(axis=(2, 4))
    ebv = np.minimum(EXP_MARGIN - pmax, 0.0).astype(np.float32)  # n,c,hp
    eb = np.ascontiguousarray(
        np.broadcast_to(ebv.reshape(NCORES, 1, CHUNKS_PER_CORE * NPAIR),
                        (NCORES, 128, CHUNKS_PER_CORE * NPAIR)))
    return xt, xdr, eb


def _run(hs, trace=False, trace_kwargs=None):
    from concourse.bass_utils import run_bass_kernel_spmd
    nc = _get_program()
    xt, xdr, eb = _prep_inputs(hs)
    in_maps = [{"xt": xt[i], "xdr": xdr[i], "eb": eb[i]}
               for i in range(NCORES)]
    return run_bass_kernel_spmd(nc, in_maps, core_ids=list(range(NCORES)),
                                trace=trace, **(trace_kwargs or {}))


def kernel(hidden_states, attention_mask):
    hs = np.ascontiguousarray(np.asarray(hidden_states, dtype=np.float32))
    mask = np.asarray(attention_mask, dtype=np.float32)
    assert hs.shape == (B, S, D)
    if mask.size and np.any(mask != 0.0):
        return _reference_numpy(hs, mask)
    res = _run(hs)
    out = np.concatenate(
        [np.asarray(res.results[i]["y"]).astype(np.float32)
         for i in range(NCORES)], axis=0)
    return out.reshape(B, S, D)


# revision 31
# speedup vs baseline: 1.0541x; 1.0541x over previous
"""Block self-attention (chunked, q=k=v, no projections) on 8 Trainium2 cores.

Math (per reference): x:[B,S,D] -> [B,H,S,dh] -> chunks of 256 along S ->
per (b,chunk,head): A = x_chunk [256,64]; S = A@A.T/8; P = softmax(S);
O = P@A -> reassembled to [B,S,D].

Device-side structure (ACT-bound design, fp16 datapath):
  * The host pre-transposes x into the [d, seq] layout the PE needs, so the
    kernel does NO on-chip input transposes of x and no PSUM->SBUF copies
    of them.
  * S is symmetric per head, so only the upper blocks [S00|S01|S11] are
    computed and exp'd on the ACT engine (3/4 of the elements; ACT is the
    bottleneck engine).  The missing lower block E10 = E01^T is recovered
    with one fp16 PE transpose per head + a DVE copy back to SBUF.
  * The PV matmul's moving operand is a host-prebuilt fp16 tile with a ones
    column appended per head; the ones column makes the PV matmul also emit
    the softmax denominator (row sum of E).
  * exp() runs as exp(score/8 + bias) with a per-(chunk,head-pair) bias
    computed on the host from row norms (Cauchy-Schwarz bound); the shift
    cancels exactly in the normalization.
  * Output is written as fp16 (halves output DMA); the host casts to fp32.
  * Engine-stream emission order per pair-iteration P:
    PE [mm1(P), mm2(P-2), E10transpose(P-1)] so the only ACT-dependent PE
    instruction sits at the END of the iteration's PE work, keeping the
    stream dense while ACT (the critical path) stays saturated.

Sharding: data-parallel over the fused (batch*chunk) dim: 64 chunks total,
8 consecutive chunks per core == one contiguous [2048, 1024] row-slice of
the flattened [16384, 1024] input per core.
"""

import numpy as np

B, S, D = 4, 4096, 1024
H = 16
DH = D // H              # 64
CHUNK = 256
NCORES = 8
NPAIR = H // 2           # 8 head pairs
ROWS_PER_CORE = (B * S) // NCORES         # 2048
CHUNKS_PER_CORE = ROWS_PER_CORE // CHUNK  # 8
SCALE = 1.0 / 8.0        # 1/sqrt(dh)
GW = DH + 1              # per-head group width in the ones-augmented operand
# exp output stays well inside fp16 range and above its subnormals:
# ln(30000) ~ 10.3 of headroom below fp16 max 65504.
EXP_MARGIN = float(np.log(30000.0))

USE_SYM = True      # block-symmetric exp (3 of 4 blocks) + E10 transpose

_PROGRAM = None


def _build_program():
    import concourse.bass as bass
    import concourse.tile as tile
    from concourse import bacc, mybir
    from concourse.masks import make_identity

    f32 = mybir.dt.float32
    f16 = mybir.dt.float16
    Exp = mybir.ActivationFunctionType.Exp

    nc = bacc.Bacc("TRN2", target_bir_lowering=False, debug=False,
                   num_devices=NCORES)
    # xt: host-transposed input. Row (c*128+p), col (hp*256+s) holds
    # x[c*256+s, (2hp + p//64)*64 + p%64] for chunk c of this core.
    xt = nc.dram_tensor("xt", [CHUNKS_PER_CORE * 128, NPAIR * CHUNK], f16,
                        kind="ExternalInput")
    # xdr: interleaved+ones PV moving operand. Row (c*128+p), col
    # (i*H*GW + h*GW + dd) holds x[c*256 + i*128 + p, h*64+dd] (dd<64) or 1.
    xdr = nc.dram_tensor("xdr", [CHUNKS_PER_CORE * 128, 2 * H * GW], f16,
                         kind="ExternalInput")
    # eb: per-(chunk, pair) exp bias, replicated across partitions.
    eb = nc.dram_tensor("eb", [128, CHUNKS_PER_CORE * NPAIR], f32,
                        kind="ExternalInput")
    y = nc.dram_tensor("y", [ROWS_PER_CORE, D], f16, kind="ExternalOutput")
    xtap = xt.ap()
    xdap = xdr.ap()
    yap = y.ap()

    # e-tile layout per head (stride 512): [B00 | B01 | B11 | B10t] (sym)
    # or [B00 | B01 | B10 | B11] (full), each block [128,128].
    with tile.TileContext(nc) as tc:
        with (
            tc.tile_pool(name="const", bufs=1) as const_pool,
            tc.tile_pool(name="xt", bufs=12) as xt_pool,
            tc.tile_pool(name="xd", bufs=3) as xd_pool,
            tc.tile_pool(name="sc", bufs=2, space="PSUM") as sc_pool,
            tc.tile_pool(name="et", bufs=2, space="PSUM") as et_pool,
            tc.tile_pool(name="o", bufs=2, space="PSUM") as o_pool,
            tc.tile_pool(name="e8", bufs=6) as e_pool,
            tc.tile_pool(name="rc", bufs=8) as r_pool,
            tc.tile_pool(name="y", bufs=3) as y_pool,
        ):
            ebias = const_pool.tile([128, CHUNKS_PER_CORE * NPAIR], f32)
            nc.sync.dma_start(out=ebias[:], in_=eb.ap())
            ident = const_pool.tile([128, 128], f16)

            def emit_front(c, hp, xt_t):
                # mm1 (upper blocks) + exp for pair (c, hp); xt_t is the
                # 2-pair [128, 512] slice tile holding pairs (hp&~1, hp|1).
                s_ps = sc_pool.tile([128, 1024], f32, tag="sc",
                                    name=f"sc{c}_{hp}")
                x0 = (hp % 2) * CHUNK
                for hi in range(2):
                    lt = xt_t[64 * hi:64 * hi + 64, :]
                    col = hi * 512
                    if USE_SYM:
                        # [S00|S01] (q0 x all k), then S11.
                        nc.tensor.matmul(
                            out=s_ps[:, col:col + 256],
                            lhsT=lt[:, x0:x0 + 128], rhs=lt[:, x0:x0 + 256],
                            start=True, stop=True)
                        nc.tensor.matmul(
                            out=s_ps[:, col + 256:col + 384],
                            lhsT=lt[:, x0 + 128:x0 + 256],
                            rhs=lt[:, x0 + 128:x0 + 256],
                            start=True, stop=True)
                    else:
                        for qm in range(2):
                            nc.tensor.matmul(
                                out=s_ps[:, col + qm * 256:col + qm * 256 + 256],
                                lhsT=lt[:, x0 + qm * 128:x0 + qm * 128 + 128],
                                rhs=lt[:, x0:x0 + 256],
                                start=True, stop=True)

                e8 = e_pool.tile([128, 1024], f16, tag="e", name=f"e{c}_{hp}")
                ncols = 384 if USE_SYM else 512
                e_out = bass.AP(tensor=e8.tensor, offset=e8.offset,
                                ap=[e8.ap[0], [512, 2], [1, ncols]])
                s_in = bass.AP(tensor=s_ps.tensor, offset=s_ps.offset,
                               ap=[s_ps.ap[0], [512, 2], [1, ncols]])
                nc.scalar.activation(out=e_out, in_=s_in, func=Exp,
                                     scale=SCALE,
                                     bias=ebias[:, c * NPAIR + hp:
                                                c * NPAIR + hp + 1])
                return e8

            def emit_mid(c, hp, e8):
                # E10 = E01^T via PE transpose, copied back into the e-tile.
                if not USE_SYM:
                    return
                et_ps = et_pool.tile([128, 256], f16, tag="et",
                                     name=f"et{c}_{hp}")
                for hi in range(2):
                    nc.tensor.matmul(
                        out=et_ps[:, hi * 128:hi * 128 + 128],
                        lhsT=e8[:, hi * 512 + 128:hi * 512 + 256],
                        rhs=ident[:], is_transpose=True,
                        start=True, stop=True)
                dst = bass.AP(tensor=e8.tensor, offset=e8.offset + 384,
                              ap=[e8.ap[0], [512, 2], [1, 128]])
                src = bass.AP(tensor=et_ps.tensor, offset=et_ps.offset,
                              ap=[et_ps.ap[0], [128, 2], [1, 128]])
                nc.vector.tensor_copy(out=dst, in_=src)

            def emit_back(c, hp, e8, xd_t, yt):
                # PV matmul (+ ones-column denominator), reciprocal,
                # normalization into the fp16 output tile, and the 2-pair
                # group's output DMA after its second pair's norm.
                o_ps = o_pool.tile([128, 4 * GW], f32, tag="o",
                                   name=f"o{c}_{hp}")
                for hi in range(2):
                    h = 2 * hp + hi
                    for b_ in range(2):
                        g = b_ * 2 + hi   # group order: b-major for norm AP
                        # stationary block for (out half b_, k half i):
                        #   sym:  i=0 -> B00/B01 (col b_*128)
                        #         i=1 -> B10t(384) / B11(256)
                        #   full: col i*256 + b_*128
                        for i in range(2):
                            if USE_SYM:
                                w_off = (hi * 512 + b_ * 128 if i == 0
                                         else hi * 512 +
                                         (384 if b_ == 0 else 256))
                            else:
                                w_off = hi * 512 + i * 256 + b_ * 128
                            nc.tensor.matmul(
                                out=o_ps[:, g * GW:(g + 1) * GW],
                                lhsT=e8[:, w_off:w_off + 128],
                                rhs=bass.AP(
                                    tensor=xd_t.tensor,
                                    offset=xd_t.offset + i * H * GW + h * GW,
                                    ap=[xd_t.ap[0], [1, GW]]),
                                start=(i == 0), stop=(i == 1))

                rc = r_pool.tile([128, 4], f32, tag="rc", name=f"rc{c}_{hp}")
                o_g = o_ps[:].rearrange("p (g c) -> p g c", c=GW)
                nc.vector.reciprocal(
                    out=rc[:].rearrange("p (g c) -> p g c", c=1),
                    in_=o_g[:, :, DH:GW])
                # yt[:, b*1024 + hp*128 + hi*64 + dd] =
                #     o_ps[:, (b*2+hi)*GW + dd] * rc[:, b*2+hi]
                out_v = bass.AP(tensor=yt.tensor,
                                offset=yt.offset + hp * 128,
                                ap=[yt.ap[0], [1024, 2], [64, 2], [1, DH]])
                in0 = bass.AP(tensor=o_ps.tensor, offset=o_ps.offset,
                              ap=[o_ps.ap[0], [2 * GW, 2], [GW, 2], [1, DH]])
                in1 = bass.AP(tensor=rc.tensor, offset=rc.offset,
                              ap=[rc.ap[0], [2, 2], [1, 2], [0, DH]])
                nc.vector.tensor_mul(out_v, in0, in1)
                if hp % 2 == 1:
                    # stream out this 2-pair column group (256 cols) so the
                    # kernel tail only waits on the last group, not a whole
                    # chunk.
                    g = hp // 2
                    dst = bass.AP(tensor=yap.tensor,
                                  offset=c * CHUNK * D + g * 256,
                                  ap=[[D, 128], [128 * D, 2], [1, 256]])
                    src = bass.AP(tensor=yt.tensor,
                                  offset=yt.offset + g * 256,
                                  ap=[yt.ap[0], [1024, 2], [1, 256]])
                    nc.sync.dma_start(out=dst, in_=src)

            # Emission order per iteration P: front(P) [PE mm1 + ACT exp],
            # back(P-2) [PE mm2 + DVE], mid(P-1) [PE transpose + DVE copy].
            # mid's transpose is the only PE instruction that waits on ACT,
            # and it sits after the iteration's other PE work.
            stages = []   # (c, hp, e8, xd_t, yt)
            lag = 2 if USE_SYM else 1
            for c in range(CHUNKS_PER_CORE):
                # Input loads ride the (otherwise idle) GpSimd sequencer so
                # Sync's ~650ns-per-dispatch budget stays with the output
                # stream.  xt is loaded as four separate 2-pair tiles: a
                # pair's mm1 then depends only on its own 128KB slice, so
                # the first exp fires ~10us earlier than with one 530KB
                # load-tile dependency.
                xtg = []
                for sl in range(4):
                    w = 2 * CHUNK
                    t = xt_pool.tile([128, w], f16, tag="xt",
                                     name=f"xt{c}_{sl}")
                    nc.gpsimd.dma_start(
                        out=t[:],
                        in_=xtap[c * 128:(c + 1) * 128, sl * w:(sl + 1) * w])
                    xtg.append(t)
                xd_t = xd_pool.tile([128, 2 * H * GW], f16, tag="xd",
                                    name=f"xd{c}")
                for sl in range(2):
                    w = H * GW
                    nc.gpsimd.dma_start(
                        out=xd_t[:, sl * w:(sl + 1) * w],
                        in_=xdap[c * 128:(c + 1) * 128, sl * w:(sl + 1) * w])
                if c == 0:
                    # identity (for the E10 transposes) is first needed a
                    # few pairs in; build it after the chunk-0 dispatches.
                    make_identity(nc, ident[:])
                yt = y_pool.tile([128, 2 * 1024], f16, tag="y", name=f"y{c}")

                for hp in range(NPAIR):
                    e8 = emit_front(c, hp, xtg[hp // 2])
                    if len(stages) >= lag:
                        emit_back(*stages[-lag])
                    if USE_SYM and len(stages) >= 1:
                        emit_mid(*stages[-1][:3])
                    stages.append((c, hp, e8, xd_t, yt))
                    stages = stages[-(lag + 1):]
            # drain
            if USE_SYM:
                emit_mid(*stages[-1][:3])
            for st in stages[-lag:]:
                emit_back(*st)

    nc.compile()
    return nc


def _get_program():
    global _PROGRAM
    if _PROGRAM is None:
        _PROGRAM = _build_program()
    return _PROGRAM


def _reference_numpy(hs, mask):
    # Exact reference math in numpy; only used if a nonzero mask ever shows
    # up (the input spec pins the mask to zeros).
    NC_ = S // CHUNK
    xx = hs.reshape(B, S, H, DH).transpose(0, 2, 1, 3)
    q = xx.reshape(B * NC_, H, CHUNK, DH)
    m = mask.reshape(B * NC_, 1, 1, CHUNK)
    scores = np.einsum('bhqd,bhkd->bhqk', q, q) / np.sqrt(DH) + m
    scores -= scores.max(axis=-1, keepdims=True)
    probs = np.exp(scores)
    probs /= probs.sum(axis=-1, keepdims=True)
    ctx = np.einsum('bhqk,bhkd->bhqd', probs, q)
    return (ctx.reshape(B, H, S, DH).transpose(0, 2, 1, 3)
            .reshape(B, S, D).astype(np.float32))


def _prep_inputs(hs):
    """Host-side layout prep: transposed fp16 operand, interleaved+ones PV
    operand, per-(chunk,pair) exp biases."""
    x16 = hs.astype(np.float16)                       # [B,S,D]
    v = x16.reshape(NCORES, CHUNKS_PER_CORE, CHUNK, H, DH)  # n,c,s,h,d
    # xt[n, c, hi*64+d, hp, s]
    xt = (v.reshape(NCORES, CHUNKS_PER_CORE, CHUNK, NPAIR, 2, DH)
          .transpose(0, 1, 4, 5, 3, 2)               # n,c,hi,d,hp,s
          .reshape(NCORES, CHUNKS_PER_CORE * 128, NPAIR * CHUNK))
    xt = np.ascontiguousarray(xt)
    # xdr[n, c, p, i, h, dd]
    w = v.reshape(NCORES, CHUNKS_PER_CORE, 2, 128, H, DH)
    aug = np.empty((NCORES, CHUNKS_PER_CORE, 2, 128, H, GW), dtype=np.float16)
    aug[..., :DH] = w
    aug[..., DH] = np.float16(1.0)
    xdr = np.ascontiguousarray(
        aug.transpose(0, 1, 3, 2, 4, 5)
        .reshape(NCORES, CHUNKS_PER_CORE * 128, 2 * H * GW))
    # per-(core, chunk, pair) bias from the Cauchy-Schwarz score bound
    n2 = (x16.astype(np.float32) ** 2).reshape(
        NCORES, CHUNKS_PER_CORE, CHUNK, H, DH).sum(-1) * SCALE  # n,c,s,h
    pmax = n2.reshape(NCORES, CHUNKS_PER_CORE, CHUNK, NPAIR, 2).max(axis=(2, 4))
    ebv = np.minimum(EXP_MARGIN - pmax, 0.0).astype(np.float32)  # n,c,hp
    eb = np.ascontiguousarray(
        np.broadcast_to(ebv.reshape(NCORES, 1, CHUNKS_PER_CORE * NPAIR),
                        (NCORES, 128, CHUNKS_PER_CORE * NPAIR)))
    return xt, xdr, eb


def _run(hs, trace=False, trace_kwargs=None):
    from concourse.bass_utils import run_bass_kernel_spmd
    nc = _get_program()
    xt, xdr, eb = _prep_inputs(hs)
    in_maps = [{"xt": xt[i], "xdr": xdr[i], "eb": eb[i]}
               for i in range(NCORES)]
    return run_bass_kernel_spmd(nc, in_maps, core_ids=list(range(NCORES)),
                                trace=trace, **(trace_kwargs or {}))


def kernel(hidden_states, attention_mask):
    hs = np.ascontiguousarray(np.asarray(hidden_states, dtype=np.float32))
    mask = np.asarray(attention_mask, dtype=np.float32)
    assert hs.shape == (B, S, D)
    if mask.size and np.any(mask != 0.0):
        return _reference_numpy(hs, mask)
    res = _run(hs)
    out = np.concatenate(
        [np.asarray(res.results[i]["y"]).astype(np.float32)
         for i in range(NCORES)], axis=0)
    return out.reshape(B, S, D)


# revision 32
# speedup vs baseline: 1.2520x; 1.1878x over previous
"""Block self-attention (chunked, q=k=v, no projections) on 8 Trainium2 cores.

Math (per reference): x:[B,S,D] -> [B,H,S,dh] -> chunks of 256 along S ->
per (b,chunk,head): A = x_chunk [256,64]; S = A@A.T/8; P = softmax(S);
O = P@A -> reassembled to [B,S,D].

Device-side structure (ACT-bound design, fp16 datapath):
  * The host pre-transposes x into the [d, seq] layout the PE needs, so the
    kernel does NO on-chip input transposes of x and no PSUM->SBUF copies
    of them.
  * S is symmetric per head, so only the upper blocks [S00|S01|S11] are
    computed and exp'd on the ACT engine (3/4 of the elements; ACT is the
    bottleneck engine).  The missing lower block E10 = E01^T is recovered
    with one fp16 PE transpose per head + a DVE copy back to SBUF.
  * The PV matmul's moving operand is a host-prebuilt fp16 tile with a ones
    column appended per head; the ones column makes the PV matmul also emit
    the softmax denominator (row sum of E).
  * exp() runs as exp(score/8 + bias) with a per-(chunk,head-pair) bias
    computed on the host from row norms (Cauchy-Schwarz bound); the shift
    cancels exactly in the normalization.
  * Output is written as fp16 (halves output DMA); the host casts to fp32.
  * Engine-stream emission order per pair-iteration P:
    PE [mm1(P), mm2(P-2), E10transpose(P-1)] so the only ACT-dependent PE
    instruction sits at the END of the iteration's PE work, keeping the
    stream dense while ACT (the critical path) stays saturated.

Sharding: data-parallel over the fused (batch*chunk) dim: 64 chunks total,
8 consecutive chunks per core == one contiguous [2048, 1024] row-slice of
the flattened [16384, 1024] input per core.
"""

import numpy as np

B, S, D = 4, 4096, 1024
H = 16
DH = D // H              # 64
CHUNK = 256
NCORES = 8
NPAIR = H // 2           # 8 head pairs
ROWS_PER_CORE = (B * S) // NCORES         # 2048
CHUNKS_PER_CORE = ROWS_PER_CORE // CHUNK  # 8
SCALE = 1.0 / 8.0        # 1/sqrt(dh)
GW = DH + 1              # per-head group width in the ones-augmented operand
# exp output stays well inside fp16 range and above its subnormals:
# ln(30000) ~ 10.3 of headroom below fp16 max 65504.
EXP_MARGIN = float(np.log(30000.0))

USE_SYM = True      # block-symmetric exp (3 of 4 blocks) + E10 transpose

_PROGRAM = None


def _build_program():
    import concourse.bass as bass
    import concourse.tile as tile
    from concourse import bacc, mybir
    from concourse.masks import make_identity

    f32 = mybir.dt.float32
    f16 = mybir.dt.float16
    Exp = mybir.ActivationFunctionType.Exp

    nc = bacc.Bacc("TRN2", target_bir_lowering=False, debug=False,
                   num_devices=NCORES)
    # xt: host-transposed input. Row (c*128+p), col (hp*256+s) holds
    # x[c*256+s, (2hp + p//64)*64 + p%64] for chunk c of this core.
    xt = nc.dram_tensor("xt", [CHUNKS_PER_CORE * 128, NPAIR * CHUNK], f16,
                        kind="ExternalInput")
    # xdr: interleaved+ones PV moving operand. Row (c*128+p), col
    # (i*H*GW + h*GW + dd) holds x[c*256 + i*128 + p, h*64+dd] (dd<64) or 1.
    xdr = nc.dram_tensor("xdr", [CHUNKS_PER_CORE * 128, 2 * H * GW], f16,
                         kind="ExternalInput")
    # eb: per-(chunk, pair) exp bias, replicated across partitions.
    eb = nc.dram_tensor("eb", [128, CHUNKS_PER_CORE * NPAIR], f32,
                        kind="ExternalInput")
    y = nc.dram_tensor("y", [ROWS_PER_CORE, D], f16, kind="ExternalOutput")
    xtap = xt.ap()
    xdap = xdr.ap()
    yap = y.ap()

    # e-tile layout per head (stride 512): [B00 | B01 | B11 | B10t] (sym)
    # or [B00 | B01 | B10 | B11] (full), each block [128,128].
    with tile.TileContext(nc) as tc:
        with (
            tc.tile_pool(name="const", bufs=1) as const_pool,
            tc.tile_pool(name="xt", bufs=3) as xt_pool,
            tc.tile_pool(name="xd", bufs=3) as xd_pool,
            tc.tile_pool(name="sc", bufs=2, space="PSUM") as sc_pool,
            tc.tile_pool(name="et", bufs=2, space="PSUM") as et_pool,
            tc.tile_pool(name="o", bufs=2, space="PSUM") as o_pool,
            tc.tile_pool(name="e8", bufs=6) as e_pool,
            tc.tile_pool(name="rc", bufs=8) as r_pool,
            tc.tile_pool(name="y", bufs=3) as y_pool,
        ):
            ebias = const_pool.tile([128, CHUNKS_PER_CORE * NPAIR], f32)
            nc.sync.dma_start(out=ebias[:], in_=eb.ap())
            ident = const_pool.tile([128, 128], f16)
            make_identity(nc, ident[:])

            def emit_front(c, hp, xt_t):
                # mm1 (upper blocks) + exp for pair (c, hp); xt_t is the
                # 2-pair [128, 512] slice tile holding pairs (hp&~1, hp|1).
                s_ps = sc_pool.tile([128, 1024], f32, tag="sc",
                                    name=f"sc{c}_{hp}")
                x0 = hp * CHUNK
                for hi in range(2):
                    lt = xt_t[64 * hi:64 * hi + 64, :]
                    col = hi * 512
                    if USE_SYM:
                        # [S00|S01] (q0 x all k), then S11.
                        nc.tensor.matmul(
                            out=s_ps[:, col:col + 256],
                            lhsT=lt[:, x0:x0 + 128], rhs=lt[:, x0:x0 + 256],
                            start=True, stop=True)
                        nc.tensor.matmul(
                            out=s_ps[:, col + 256:col + 384],
                            lhsT=lt[:, x0 + 128:x0 + 256],
                            rhs=lt[:, x0 + 128:x0 + 256],
                            start=True, stop=True)
                    else:
                        for qm in range(2):
                            nc.tensor.matmul(
                                out=s_ps[:, col + qm * 256:col + qm * 256 + 256],
                                lhsT=lt[:, x0 + qm * 128:x0 + qm * 128 + 128],
                                rhs=lt[:, x0:x0 + 256],
                                start=True, stop=True)

                e8 = e_pool.tile([128, 1024], f16, tag="e", name=f"e{c}_{hp}")
                ncols = 384 if USE_SYM else 512
                e_out = bass.AP(tensor=e8.tensor, offset=e8.offset,
                                ap=[e8.ap[0], [512, 2], [1, ncols]])
                s_in = bass.AP(tensor=s_ps.tensor, offset=s_ps.offset,
                               ap=[s_ps.ap[0], [512, 2], [1, ncols]])
                nc.scalar.activation(out=e_out, in_=s_in, func=Exp,
                                     scale=SCALE,
                                     bias=ebias[:, c * NPAIR + hp:
                                                c * NPAIR + hp + 1])
                return e8

            def emit_mid(c, hp, e8):
                # E10 = E01^T via PE transpose, copied back into the e-tile.
                if not USE_SYM:
                    return
                et_ps = et_pool.tile([128, 256], f16, tag="et",
                                     name=f"et{c}_{hp}")
                for hi in range(2):
                    nc.tensor.matmul(
                        out=et_ps[:, hi * 128:hi * 128 + 128],
                        lhsT=e8[:, hi * 512 + 128:hi * 512 + 256],
                        rhs=ident[:], is_transpose=True,
                        start=True, stop=True)
                dst = bass.AP(tensor=e8.tensor, offset=e8.offset + 384,
                              ap=[e8.ap[0], [512, 2], [1, 128]])
                src = bass.AP(tensor=et_ps.tensor, offset=et_ps.offset,
                              ap=[et_ps.ap[0], [128, 2], [1, 128]])
                nc.vector.tensor_copy(out=dst, in_=src)

            def emit_back(c, hp, e8, xd_t, yt):
                # PV matmul (+ ones-column denominator), reciprocal,
                # normalization into the fp16 output tile, and the 2-pair
                # group's output DMA after its second pair's norm.
                o_ps = o_pool.tile([128, 4 * GW], f32, tag="o",
                                   name=f"o{c}_{hp}")
                for hi in range(2):
                    h = 2 * hp + hi
                    for b_ in range(2):
                        g = b_ * 2 + hi   # group order: b-major for norm AP
                        # stationary block for (out half b_, k half i):
                        #   sym:  i=0 -> B00/B01 (col b_*128)
                        #         i=1 -> B10t(384) / B11(256)
                        #   full: col i*256 + b_*128
                        for i in range(2):
                            if USE_SYM:
                                w_off = (hi * 512 + b_ * 128 if i == 0
                                         else hi * 512 +
                                         (384 if b_ == 0 else 256))
                            else:
                                w_off = hi * 512 + i * 256 + b_ * 128
                            nc.tensor.matmul(
                                out=o_ps[:, g * GW:(g + 1) * GW],
                                lhsT=e8[:, w_off:w_off + 128],
                                rhs=bass.AP(
                                    tensor=xd_t.tensor,
                                    offset=xd_t.offset + i * H * GW + h * GW,
                                    ap=[xd_t.ap[0], [1, GW]]),
                                start=(i == 0), stop=(i == 1))

                rc = r_pool.tile([128, 4], f32, tag="rc", name=f"rc{c}_{hp}")
                o_g = o_ps[:].rearrange("p (g c) -> p g c", c=GW)
                nc.vector.reciprocal(
                    out=rc[:].rearrange("p (g c) -> p g c", c=1),
                    in_=o_g[:, :, DH:GW])
                # yt[:, b*1024 + hp*128 + hi*64 + dd] =
                #     o_ps[:, (b*2+hi)*GW + dd] * rc[:, b*2+hi]
                out_v = bass.AP(tensor=yt.tensor,
                                offset=yt.offset + hp * 128,
                                ap=[yt.ap[0], [1024, 2], [64, 2], [1, DH]])
                in0 = bass.AP(tensor=o_ps.tensor, offset=o_ps.offset,
                              ap=[o_ps.ap[0], [2 * GW, 2], [GW, 2], [1, DH]])
                in1 = bass.AP(tensor=rc.tensor, offset=rc.offset,
                              ap=[rc.ap[0], [2, 2], [1, 2], [0, DH]])
                nc.vector.tensor_mul(out_v, in0, in1)
                if hp % 2 == 1:
                    # stream out this 2-pair column group (256 cols) so the
                    # kernel tail only waits on the last group, not a whole
                    # chunk.
                    g = hp // 2
                    dst = bass.AP(tensor=yap.tensor,
                                  offset=c * CHUNK * D + g * 256,
                                  ap=[[D, 128], [128 * D, 2], [1, 256]])
                    src = bass.AP(tensor=yt.tensor,
                                  offset=yt.offset + g * 256,
                                  ap=[yt.ap[0], [1024, 2], [1, 256]])
                    nc.sync.dma_start(out=dst, in_=src)

            # Emission order per iteration P: front(P) [PE mm1 + ACT exp],
            # back(P-2) [PE mm2 + DVE], mid(P-1) [PE transpose + DVE copy].
            # mid's transpose is the only PE instruction that waits on ACT,
            # and it sits after the iteration's other PE work.
            stages = []   # (c, hp, e8, xd_t, yt)
            lag = 2 if USE_SYM else 1
            for c in range(CHUNKS_PER_CORE):
                # Input loads ride the (otherwise idle) GpSimd sequencer so
                # Sync's ~650ns-per-dispatch budget stays with the output
                # stream.  xt is loaded as four separate 2-pair tiles: a
                # pair's mm1 then depends only on its own 128KB slice, so
                # the first exp fires ~10us earlier than with one 530KB
                # load-tile dependency.
                xt_t = xt_pool.tile([128, NPAIR * CHUNK], f16, tag="xt",
                                    name=f"xt{c}")
                for sl in range(4):
                    w = NPAIR * CHUNK // 4
                    nc.gpsimd.dma_start(
                        out=xt_t[:, sl * w:(sl + 1) * w],
                        in_=xtap[c * 128:(c + 1) * 128, sl * w:(sl + 1) * w])
                xd_t = xd_pool.tile([128, 2 * H * GW], f16, tag="xd",
                                    name=f"xd{c}")
                for sl in range(2):
                    w = H * GW
                    nc.gpsimd.dma_start(
                        out=xd_t[:, sl * w:(sl + 1) * w],
                        in_=xdap[c * 128:(c + 1) * 128, sl * w:(sl + 1) * w])
                yt = y_pool.tile([128, 2 * 1024], f16, tag="y", name=f"y{c}")

                for hp in range(NPAIR):
                    e8 = emit_front(c, hp, xt_t)
                    if len(stages) >= lag:
                        emit_back(*stages[-lag])
                    if USE_SYM and len(stages) >= 1:
                        emit_mid(*stages[-1][:3])
                    stages.append((c, hp, e8, xd_t, yt))
                    stages = stages[-(lag + 1):]
            # drain
            if USE_SYM:
                emit_mid(*stages[-1][:3])
            for st in stages[-lag:]:
                emit_back(*st)

    nc.compile()
    return nc


def _get_program():
    global _PROGRAM
    if _PROGRAM is None:
        _PROGRAM = _build_program()
    return _PROGRAM


def _reference_numpy(hs, mask):
    # Exact reference math in numpy; only used if a nonzero mask ever shows
    # up (the input spec pins the mask to zeros).
    NC_ = S // CHUNK
    xx = hs.reshape(B, S, H, DH).transpose(0, 2, 1, 3)
    q = xx.reshape(B * NC_, H, CHUNK, DH)
    m = mask.reshape(B * NC_, 1, 1, CHUNK)
    scores = np.einsum('bhqd,bhkd->bhqk', q, q) / np.sqrt(DH) + m
    scores -= scores.max(axis=-1, keepdims=True)
    probs = np.exp(scores)
    probs /= probs.sum(axis=-1, keepdims=True)
    ctx = np.einsum('bhqk,bhkd->bhqd', probs, q)
    return (ctx.reshape(B, H, S, DH).transpose(0, 2, 1, 3)
            .reshape(B, S, D).astype(np.float32))


def _prep_inputs(hs):
    """Host-side layout prep: transposed fp16 operand, interleaved+ones PV
    operand, per-(chunk,pair) exp biases."""
    x16 = hs.astype(np.float16)                       # [B,S,D]
    v = x16.reshape(NCORES, CHUNKS_PER_CORE, CHUNK, H, DH)  # n,c,s,h,d
    # xt[n, c, hi*64+d, hp, s]
    xt = (v.reshape(NCORES, CHUNKS_PER_CORE, CHUNK, NPAIR, 2, DH)
          .transpose(0, 1, 4, 5, 3, 2)               # n,c,hi,d,hp,s
          .reshape(NCORES, CHUNKS_PER_CORE * 128, NPAIR * CHUNK))
    xt = np.ascontiguousarray(xt)
    # xdr[n, c, p, i, h, dd]
    w = v.reshape(NCORES, CHUNKS_PER_CORE, 2, 128, H, DH)
    aug = np.empty((NCORES, CHUNKS_PER_CORE, 2, 128, H, GW), dtype=np.float16)
    aug[..., :DH] = w
    aug[..., DH] = np.float16(1.0)
    xdr = np.ascontiguousarray(
        aug.transpose(0, 1, 3, 2, 4, 5)
        .reshape(NCORES, CHUNKS_PER_CORE * 128, 2 * H * GW))
    # per-(core, chunk, pair) bias from the Cauchy-Schwarz score bound
    n2 = (x16.astype(np.float32) ** 2).reshape(
        NCORES, CHUNKS_PER_CORE, CHUNK, H, DH).sum(-1) * SCALE  # n,c,s,h
    pmax = n2.reshape(NCORES, CHUNKS_PER_CORE, CHUNK, NPAIR, 2).max(axis=(2, 4))
    ebv = np.minimum(EXP_MARGIN - pmax, 0.0).astype(np.float32)  # n,c,hp
    eb = np.ascontiguousarray(
        np.broadcast_to(ebv.reshape(NCORES, 1, CHUNKS_PER_CORE * NPAIR),
                        (NCORES, 128, CHUNKS_PER_CORE * NPAIR)))
    return xt, xdr, eb


def _run(hs, trace=False, trace_kwargs=None):
    from concourse.bass_utils import run_bass_kernel_spmd
    nc = _get_program()
    xt, xdr, eb = _prep_inputs(hs)
    in_maps = [{"xt": xt[i], "xdr": xdr[i], "eb": eb[i]}
               for i in range(NCORES)]
    return run_bass_kernel_spmd(nc, in_maps, core_ids=list(range(NCORES)),
                                trace=trace, **(trace_kwargs or {}))


def kernel(hidden_states, attention_mask):
    hs = np.ascontiguousarray(np.asarray(hidden_states, dtype=np.float32))
    mask = np.asarray(attention_mask, dtype=np.float32)
    assert hs.shape == (B, S, D)
    if mask.size and np.any(mask != 0.0):
        return _reference_numpy(hs, mask)
    res = _run(hs)
    out = np.concatenate(
        [np.asarray(res.results[i]["y"]).astype(np.float32)
         for i in range(NCORES)], axis=0)
    return out.reshape(B, S, D)
